# revision 26
# baseline (speedup 1.0000x reference)
"""MDGRec GNN message-passing kernel for 8 Trainium2 NeuronCores.

Strategy (SPMD, one NEFF on 8 cores):
  - Nodes row-sharded: core m owns dst rows [m*18750, (m+1)*18750).
  - Host relabels nodes with a permutation pi so that each core's bin-packed
    128-row groups occupy contiguous rows of a padded 19200-row shard; all
    device-side writes/reads become contiguous slice DMAs.
  - id and text features concatenated into 128-wide rows; the 1/(N_LAYERS+1)
    layer-mean factor is folded into the staged id/text weights so every
    propagated term is pre-divided by 3.
  - Layer tables (full [153600, 128] bf16 in pi-space) built via on-device
    AllGather between layers.
  - SpMM per layer: bulk dma_gather of h[edge_col] striped across the 4 SWDGE
    queues (descriptor generation parallelizes across Q7 core pairs), scatter
    matrices built as two fused wide DVE ops per supergroup (is_eq + val
    multiply over [128, 70, 128]), segment-sum via PE matmuls into PSUM.
  - Fused epilogue (tail amp, gate, blend) computed per supergroup in
    transposed space; h0/h1 re-added on the PE via identity matmuls.
"""

import os
import numpy as np
import ml_dtypes

import concourse.bass as bass
import concourse.bacc as bacc
import concourse.tile as tile
import concourse.mybir as mybir
from concourse import bass_utils, library_config
from concourse.masks import make_identity

# ---- problem constants (hardcoded per spec) ----
N_NODES = 150000
EMB_DIM = 64
TEXT_DIM = 384
NCORES = 8
SHARD = N_NODES // NCORES          # 18750 real rows per core
F = 2 * EMB_DIM                    # 128 concat feature width

# ---- template constants ----
G = 150                            # groups per core
S_G = 2                            # groups per supergroup
N_SG = G // S_G                    # 75
SHARD_P = G * 128                  # 19200 padded rows per core (pi-space)
TBL_ROWS = NCORES * SHARD_P        # 153600 pi-space nodes
N_RANGE = 5
RANGE_SIZE = TBL_ROWS // N_RANGE   # 30720 (int16-safe)
C_GR = 7                           # chunks per (group, range)
CPG = N_RANGE * C_GR               # 35 chunks per group
C_SG = S_G * CPG                   # 70 chunks per supergroup
CALL_CH = S_G * C_GR               # 14 chunks per gather call
CALL_IDX = CALL_CH * 128           # 1792 idxs per gather call
W16 = CALL_IDX // 16               # 112 idx columns per range
CAP_R = C_GR * 128                 # 896 edge capacity per (group, range)
PAD_SLOT = 999.0
N_QUEUES = int(os.environ.get("KV2_QUEUES", "4"))
NEGTRIM = os.environ.get("KV2_NEGTRIM", "0") == "1"
FUSED_S = os.environ.get("KV2_FUSED_S", "1") == "1"
ACT_MOD = int(os.environ.get("KV2_ACT_MOD", "2"))  # sg % ACT_MOD -> ACT val-mult
SINGLE_PACKET = os.environ.get("KV2_SP", "0") == "1"

_CACHE = {}
_LAST_IN_MAPS = None


# ======================================================================
# device program
# ======================================================================

def _build(single_core=False):
    fp32 = mybir.dt.float32
    bf16 = mybir.dt.bfloat16
    i16 = mybir.dt.int16

    do_collectives = not single_core
    nc = bacc.Bacc("TRN2", target_bir_lowering=False, debug=False,
                   num_devices=1 if single_core else NCORES,
                   num_swdge_queues=N_QUEUES)

    # inputs (per core)
    text_T = nc.dram_tensor("text_T", [TEXT_DIM, SHARD_P], bf16, kind="ExternalInput")
    id_b = nc.dram_tensor("id_b", [SHARD_P, EMB_DIM], bf16, kind="ExternalInput")
    gidx = nc.dram_tensor("gidx", [N_SG, 128, N_RANGE * W16], i16,
                          kind="ExternalInput")
    auxb = nc.dram_tensor("auxb", [128, N_SG * C_SG], bf16,
                          kind="ExternalInput")
    auxv = nc.dram_tensor("auxv", [128, N_SG * C_SG], fp32,
                          kind="ExternalInput")
    tailf_d = nc.dram_tensor("tailf_d", [128, N_SG * S_G], fp32,
                             kind="ExternalInput")
    w_text = nc.dram_tensor("w_text", [128, 3 * EMB_DIM], bf16, kind="ExternalInput")
    b_text = nc.dram_tensor("b_text", [128, EMB_DIM], bf16, kind="ExternalInput")
    w_fuse = nc.dram_tensor("w_fuse", [F, EMB_DIM], bf16, kind="ExternalInput")
    b_fuse = nc.dram_tensor("b_fuse", [EMB_DIM, 1], fp32, kind="ExternalInput")
    iota_d = nc.dram_tensor("iota_d", [128, 128], bf16, kind="ExternalInput")

    out = nc.dram_tensor("out", [SHARD_P, EMB_DIM], fp32, kind="ExternalOutput")

    # internal DRAM
    cat_bf = nc.dram_tensor("cat_bf", [SHARD_P, F], bf16)
    h1_bf = nc.dram_tensor("h1_bf", [SHARD_P, F], bf16)
    table0 = nc.dram_tensor("table0", [TBL_ROWS, F], bf16, addr_space="Shared")
    table1 = nc.dram_tensor("table1", [TBL_ROWS, F], bf16, addr_space="Shared")

    with tile.TileContext(nc) as tc:
        nc.gpsimd.load_library(library_config.mlp)
        with (
            tc.tile_pool(name="const", bufs=1) as cpool,
            tc.tile_pool(name="sb", bufs=3) as sb,
            tc.tile_pool(name="gx", bufs=3) as gx,
            tc.tile_pool(name="xp", bufs=3) as xp,
            tc.tile_pool(name="sp", bufs=3) as spool,
            tc.tile_pool(name="ep", bufs=2) as ep,
            tc.tile_pool(name="psum", bufs=1, space="PSUM") as ps,
            tc.tile_pool(name="psproj", bufs=2, space="PSUM") as psj,
            tc.tile_pool(name="psacc", bufs=2, space="PSUM") as psa,
        ):
            # ---- constants ----
            iota_b = cpool.tile([128, 128], bf16, tag="iota")
            nc.sync.dma_start(iota_b[:], iota_d[:])
            ident = cpool.tile([128, 128], fp32, tag="ident")
            make_identity(nc, ident[:])
            identb = cpool.tile([128, 128], bf16, tag="identb")
            nc.vector.tensor_copy(identb[:], ident[:])
            wt_t = cpool.tile([128, 3 * EMB_DIM], bf16, tag="wt")
            nc.sync.dma_start(wt_t[:], w_text[:])
            bt_t = cpool.tile([128, EMB_DIM], bf16, tag="bt")
            nc.sync.dma_start(bt_t[:], b_text[:])
            wf_t = cpool.tile([128, EMB_DIM], bf16, tag="wf")
            nc.sync.dma_start(wf_t[:], w_fuse[:])
            bf_t = cpool.tile([EMB_DIM, 1], fp32, tag="bf")
            nc.sync.dma_start(bf_t[:], b_fuse[:])
            aux_t = cpool.tile([128, N_SG * C_SG], bf16, tag="aux")
            nc.sync.dma_start(aux_t[:], auxb[:])
            auxv_t = cpool.tile([128, N_SG * C_SG], fp32, tag="auxv")
            nc.sync.dma_start(auxv_t[:], auxv[:])
            tailf_t = cpool.tile([128, N_SG, S_G], fp32, tag="tailf")
            nc.sync.dma_start(tailf_t[:].rearrange("p a b -> p (a b)"), tailf_d[:])

            # ---- text projection + cat assembly (pi-layout, all bf16) ----
            for i in range(G):
                r0 = i * 128
                proj_ps = psj.tile([128, EMB_DIM], fp32, tag="mm")
                tx3 = sb.tile([128, 3, 128], bf16, tag="tx3")
                for k in range(3):
                    nc.sync.dma_start(tx3[:, k, :],
                                      text_T[k * 128:(k + 1) * 128, r0:r0 + 128])
                for k in range(3):
                    nc.tensor.matmul(proj_ps[:], lhsT=tx3[:, k, :],
                                     rhs=wt_t[:, k * EMB_DIM:(k + 1) * EMB_DIM],
                                     start=(k == 0), stop=(k == 2))
                catb = sb.tile([128, F], bf16, tag="catb")
                nc.sync.dma_start(catb[:, 0:EMB_DIM], id_b[r0:r0 + 128, :])
                nc.vector.tensor_tensor(out=catb[:, EMB_DIM:F],
                                        in0=proj_ps[:], in1=bt_t[:],
                                        op=mybir.AluOpType.add)
                nc.sync.dma_start(cat_bf[r0:r0 + 128, :], catb[:])

            # ---- AllGather h0 ----
            if do_collectives:
                nc.gpsimd.collective_compute(
                    "AllGather", mybir.AluOpType.bypass,
                    replica_groups=[list(range(NCORES))],
                    ins=[cat_bf[:]],
                    outs=[table0[:]],
                )

            # ---- SpMM layers ----
            nidx_reg = nc.gpsimd.to_reg(CALL_IDX)
            qc = 0
            for layer in (0, 1):
                table = table0 if layer == 0 else table1
                for sg in range(N_SG):
                    gi = gx.tile([128, N_RANGE * W16], i16, tag="gi")
                    nc.sync.dma_start(gi[:], gidx[sg, :, :])
                    if layer == 1:
                        h0t = sb.tile([128, S_G, F], bf16, tag="h0")
                        h1t = sb.tile([128, S_G, F], bf16, tag="h1")
                        for s in range(S_G):
                            r0 = (sg * S_G + s) * 128
                            nc.sync.dma_start(h0t[:, s, :],
                                              cat_bf[r0:r0 + 128, :])
                            nc.sync.dma_start(h1t[:, s, :],
                                              h1_bf[r0:r0 + 128, :])

                    Xsr = []
                    for r in range(N_RANGE):
                        X = xp.tile([128, CALL_CH, F], bf16, tag=f"X{r}")
                        nc.gpsimd.dma_gather(
                            X[:],
                            table[r * RANGE_SIZE:(r + 1) * RANGE_SIZE, :],
                            gi[:, r * W16:(r + 1) * W16], CALL_IDX, nidx_reg, F,
                            single_packet=SINGLE_PACKET, queue_num=qc % N_QUEUES)
                        qc += 1
                        Xsr.append(X)

                    # S[p, ci, j] = (iota[j] == slot[p, ci]) * val[p, ci]
                    a0 = sg * C_SG
                    slot_ap = aux_t[:, a0:a0 + C_SG]
                    val_ap = auxv_t[:, a0:a0 + C_SG]
                    S_t = spool.tile([128, C_SG, 128], bf16, tag="S")
                    nc.vector.tensor_tensor(
                        out=S_t[:],
                        in0=iota_b[:, None, :].broadcast_to([128, C_SG, 128]),
                        in1=slot_ap[:, :, None].broadcast_to([128, C_SG, 128]),
                        op=mybir.AluOpType.is_equal)
                    if sg % ACT_MOD == ACT_MOD - 1:
                        # val multiply on the (otherwise idle) scalar engine
                        for ci in range(C_SG):
                            nc.scalar.activation(
                                S_t[:, ci, :], S_t[:, ci, :],
                                mybir.ActivationFunctionType.Copy,
                                scale=val_ap[:, ci:ci + 1])
                    else:
                        nc.vector.tensor_tensor(
                            out=S_t[:],
                            in0=S_t[:],
                            in1=val_ap[:, :, None].broadcast_to([128, C_SG, 128]),
                            op=mybir.AluOpType.mult)

                    acc = psa.tile([128, S_G, F], fp32, tag="acc")
                    for s in range(S_G):
                        g = sg * S_G + s
                        r0 = g * 128
                        chunks = [(r, s * C_GR + c)
                                  for r in range(N_RANGE) for c in range(C_GR)]
                        n_mm = CPG + (2 if layer == 1 else 0)
                        for j, (r, k) in enumerate(chunks):
                            ci = r * CALL_CH + k
                            nc.tensor.matmul(acc[:, s, :], lhsT=S_t[:, ci, :],
                                             rhs=Xsr[r][:, k, :],
                                             start=(j == 0), stop=(j == n_mm - 1))
                        if layer == 1:
                            nc.tensor.matmul(acc[:, s, :], lhsT=identb[:],
                                             rhs=h0t[:, s, :], start=False,
                                             stop=False)
                            nc.tensor.matmul(acc[:, s, :], lhsT=identb[:],
                                             rhs=h1t[:, s, :], start=False,
                                             stop=True)

                    if layer == 0:
                        resb = sb.tile([128, S_G, F], bf16, tag="resb")
                        nc.scalar.activation(resb[:], acc[:],
                                             mybir.ActivationFunctionType.Copy)
                        for s in range(S_G):
                            r0 = (sg * S_G + s) * 128
                            nc.sync.dma_start(h1_bf[r0:r0 + 128, :],
                                              resb[:, s, :])
                        continue

                    # ---- fused epilogue for this supergroup (node-major) ----
                    fs = ep.tile([128, S_G, F], fp32, tag="fs")
                    nc.vector.tensor_copy(fs[:], acc[:])
                    # amp on text half, per-node (per-partition) multiplier
                    nc.vector.tensor_tensor(
                        out=fs[:, :, EMB_DIM:F],
                        in0=fs[:, :, EMB_DIM:F],
                        in1=tailf_t[:, sg, :, None].broadcast_to(
                            [128, S_G, EMB_DIM]),
                        op=mybir.AluOpType.mult)
                    # transpose fsum for the gate matmul
                    tp = ps.tile([128, S_G, 128], fp32, tag="tp")
                    for s in range(S_G):
                        nc.tensor.transpose(out=tp[:, s, :], in_=fs[:, s, :],
                                            identity=ident[:])
                    ft = ep.tile([128, S_G, 128], bf16, tag="ft")
                    nc.vector.tensor_copy(ft[:], tp[:])
                    gp = ps.tile([EMB_DIM, S_G * 128], fp32, tag="gp")
                    nc.tensor.matmul(gp[:], lhsT=wf_t[:],
                                     rhs=ft[:].rearrange("p s n -> p (s n)"),
                                     start=True, stop=True)
                    gateT = ep.tile([EMB_DIM, S_G * 128], fp32, tag="gateT")
                    nc.scalar.activation(gateT[:], gp[:],
                                         mybir.ActivationFunctionType.Sigmoid,
                                         bias=bf_t[:, :1])
                    g2 = ps.tile([128, S_G, EMB_DIM], fp32, tag="g2")
                    for s in range(S_G):
                        nc.tensor.transpose(out=g2[:, s, :],
                                            in_=gateT[:, s * 128:(s + 1) * 128],
                                            identity=ident[0:EMB_DIM, 0:EMB_DIM])
                    gate = ep.tile([128, S_G, EMB_DIM], fp32, tag="gate")
                    nc.vector.tensor_copy(gate[:], g2[:])
                    # blend: fused = text + gate*(id - text)
                    dif = ep.tile([128, S_G, EMB_DIM], fp32, tag="dif")
                    nc.vector.tensor_tensor(out=dif[:], in0=fs[:, :, 0:EMB_DIM],
                                            in1=fs[:, :, EMB_DIM:F],
                                            op=mybir.AluOpType.subtract)
                    nc.vector.tensor_tensor(out=dif[:], in0=dif[:], in1=gate[:],
                                            op=mybir.AluOpType.mult)
                    res = ep.tile([128, S_G, EMB_DIM], fp32, tag="res")
                    nc.vector.tensor_tensor(out=res[:], in0=fs[:, :, EMB_DIM:F],
                                            in1=dif[:], op=mybir.AluOpType.add)
                    for s in range(S_G):
                        r0 = (sg * S_G + s) * 128
                        nc.sync.dma_start(out[r0:r0 + 128, :], res[:, s, :])

                if layer == 0 and do_collectives:
                    nc.gpsimd.collective_compute(
                        "AllGather", mybir.AluOpType.bypass,
                        replica_groups=[list(range(NCORES))],
                        ins=[h1_bf[:]],
                        outs=[table1[:]],
                    )

    nc.compile()
    return nc


# ======================================================================
# host preprocessing
# ======================================================================

def _preprocess(edge_row, edge_col, edge_val, tail_mask, amp):
    """Full host-side preprocessing. Two passes:
    1. per-core packing of dst rows into groups by total degree (snake on
       sorted degrees), defining the pi permutation; then exact per-
       (group, src-range) bucket counts are checked against CAP_R and
       repaired by moving rows between groups.
    2. per-core edge template fill (gather indices, slots, values)."""
    # ---- pass 1: pack by total degree, then repair ----
    grp = np.empty(N_NODES, np.int64)
    slot = np.empty(N_NODES, np.int64)
    deg_t = np.bincount(edge_row, minlength=N_NODES)
    for m in range(NCORES):
        lo = m * SHARD
        dt_ = deg_t[lo:lo + SHARD]
        order = np.argsort(-dt_, kind="stable")
        gassign = np.empty(SHARD, np.int64)
        idx = np.arange(SHARD)
        rounds = idx // G
        posr = idx % G
        fwd = (rounds % 2 == 0)
        gassign[order] = np.where(fwd, posr, G - 1 - posr)
        grp[lo:lo + SHARD] = gassign
        o2 = np.lexsort((np.arange(SHARD), gassign))
        sg_sorted = gassign[o2]
        starts = np.searchsorted(sg_sorted, np.arange(G))
        sl = np.arange(SHARD) - starts[sg_sorted]
        slot_l = np.empty(SHARD, np.int64)
        slot_l[o2] = sl
        slot[lo:lo + SHARD] = slot_l

    pi = ((np.arange(N_NODES) // SHARD) * SHARD_P + grp * 128 + slot)

    # ---- check/repair (group, range) capacities per core ----
    pc = pi[edge_col]
    rng_id = pc // RANGE_SIZE
    for m in range(NCORES):
        lo = m * SHARD
        sel = (edge_row >= lo) & (edge_row < lo + SHARD)
        er = edge_row[sel] - lo
        rr = rng_id[sel]
        gg = grp[lo + er]
        for _ in range(50):
            cnt = np.zeros((G, N_RANGE), np.int64)
            np.add.at(cnt, (gg, rr), 1)
            over = np.argwhere(cnt > CAP_R)
            if len(over) == 0:
                break
            nrows = np.bincount(grp[lo:lo + SHARD], minlength=G)
            deg_gr = np.zeros((SHARD, N_RANGE), np.int64)
            np.add.at(deg_gr, (er, rr), 1)
            for g_o, r_o in over:
                rows_g = np.where(grp[lo:lo + SHARD] == g_o)[0]
                rows_g = rows_g[np.argsort(-deg_gr[rows_g, r_o])]
                moved = False
                need = cnt[g_o, r_o] - CAP_R
                for row in rows_g:
                    if deg_gr[row, r_o] == 0:
                        break
                    for g_n in np.argsort(cnt[:, r_o]):
                        if g_n == g_o or nrows[g_n] >= 128:
                            continue
                        if np.all(cnt[g_n] + deg_gr[row] <= CAP_R):
                            cnt[g_o] -= deg_gr[row]
                            cnt[g_n] += deg_gr[row]
                            nrows[g_o] -= 1
                            nrows[g_n] += 1
                            grp[lo + row] = g_n
                            gg = grp[lo + er]
                            moved = True
                            break
                    need = cnt[g_o, r_o] - CAP_R
                    if need <= 0:
                        break
                if not moved and cnt[g_o, r_o] > CAP_R:
                    raise RuntimeError("capacity repair failed")
            gassign = grp[lo:lo + SHARD]
            o2 = np.lexsort((np.arange(SHARD), gassign))
            sg_sorted = gassign[o2]
            starts = np.searchsorted(sg_sorted, np.arange(G))
            sl = np.arange(SHARD) - starts[sg_sorted]
            slot_l = np.empty(SHARD, np.int64)
            slot_l[o2] = sl
            slot[lo:lo + SHARD] = slot_l
        else:
            raise RuntimeError("repair loop did not converge")
        pi = (np.arange(N_NODES) // SHARD) * SHARD_P + grp * 128 + slot
        pc = pi[edge_col]
        rng_id = pc // RANGE_SIZE

    # ---- pass 2: per-core template fill ----
    cores = []
    for m in range(NCORES):
        lo = m * SHARD
        sel = (edge_row >= lo) & (edge_row < lo + SHARD)
        er = edge_row[sel] - lo
        ev = edge_val[sel].astype(np.float32)
        e_pc = pc[sel]                      # pi-space col
        e_r = (e_pc // RANGE_SIZE).astype(np.int64)
        e_cloc = (e_pc - e_r * RANGE_SIZE).astype(np.int64)
        e_g = grp[lo + er]
        e_slot = slot[lo + er]

        bucket = e_g * N_RANGE + e_r
        eorder = np.argsort(bucket, kind="stable")
        b_sorted = bucket[eorder]
        cnt = np.bincount(b_sorted, minlength=G * N_RANGE)
        assert cnt.max() <= CAP_R, cnt.max()
        off = np.zeros(G * N_RANGE + 1, np.int64)
        np.cumsum(cnt, out=off[1:])
        pos = np.arange(len(eorder)) - off[b_sorted]

        so_g = e_g[eorder]
        so_r = e_r[eorder]
        e_sg = so_g // S_G
        e_s = so_g % S_G
        e_c = pos // 128
        e_p = pos % 128
        e_ci = so_r * CALL_CH + e_s * C_GR + e_c

        # slot aux (bf16) and val aux (fp32): [128, N_SG * C_SG]
        slot_arr = np.full((128, N_SG, C_SG), PAD_SLOT, np.float32)
        val_arr = np.zeros((128, N_SG, C_SG), np.float32)
        lin = (e_p * N_SG + e_sg) * C_SG + e_ci
        slot_arr.reshape(-1)[lin] = e_slot[eorder].astype(np.float32)
        val_arr.reshape(-1)[lin] = ev[eorder]
        aux_bf = slot_arr.reshape(128, -1).astype(ml_dtypes.bfloat16)
        aux_v = val_arr.reshape(128, -1)

        # gather indices, wrapped [16, W16] and replicated to 128 partitions;
        # tail of each (sg, range) call marked -1 (ucode trims trailing negs)
        gidx16 = np.zeros((N_SG, N_RANGE, 16, W16), np.int16)
        e_k = e_s * C_GR + e_c
        q = e_k * 128 + e_p
        lin2 = ((e_sg * N_RANGE + so_r) * 16 + (q % 16)) * W16 + (q // 16)
        gidx16.reshape(-1)[lin2] = e_cloc[eorder].astype(np.int16)
        if NEGTRIM:
            # mark trailing unused q positions of each call as -1
            qmax = np.zeros((N_SG, N_RANGE), np.int64)
            np.maximum.at(qmax, (e_sg, so_r), q + 1)
            qgrid = np.arange(CALL_IDX).reshape(16, W16, order="F")
            trail = qgrid[None, None, :, :] >= qmax[:, :, None, None]
            gidx16[trail] = -1
        gidx_arr = np.tile(gidx16, (1, 1, 8, 1))          # [N_SG, 5, 128, W16]
        gidx_arr = np.ascontiguousarray(
            gidx_arr.transpose(0, 2, 1, 3).reshape(N_SG, 128, -1))

        pi_l = grp[lo:lo + SHARD] * 128 + slot[lo:lo + SHARD]  # local padded pos
        # tail factor per node (amp or 1), node-major layout [128, N_SG*S_G]
        tf_p = np.full(SHARD_P, 1.0, np.float32)
        tmask = tail_mask[lo:lo + SHARD].astype(bool)
        tf_p[pi_l] = np.where(tmask, amp, 1.0).astype(np.float32)
        # tf_p index = (sg*S_G+s)*128 + p  ->  [p, sg*S_G+s]
        tailf = np.ascontiguousarray(tf_p.reshape(N_SG * S_G, 128).T)

        cores.append({
            "gidx": gidx_arr, "auxb": aux_bf, "auxv": aux_v, "tailf": tailf,
            "pi_l": pi_l,
        })
    return cores


def kernel(text_feats, edge_row, edge_col, edge_val, tail_mask, user_emb,
           item_emb, W_text, b_text, W_fuse, b_fuse, tail_amp):
    text_feats = np.asarray(text_feats, np.float32)
    edge_row = np.asarray(edge_row).astype(np.int64)
    edge_col = np.asarray(edge_col).astype(np.int64)
    edge_val = np.asarray(edge_val, np.float32)
    tail_mask = np.asarray(tail_mask).astype(bool)
    user_emb = np.asarray(user_emb, np.float32)
    item_emb = np.asarray(item_emb, np.float32)
    W_text = np.asarray(W_text, np.float32)
    b_text = np.asarray(b_text, np.float32)
    W_fuse = np.asarray(W_fuse, np.float32)
    b_fuse = np.asarray(b_fuse, np.float32)
    amp = float(1.0 + 1.0 / (1.0 + np.exp(-np.float64(np.asarray(tail_amp)))))

    emb_id = np.concatenate([user_emb, item_emb], axis=0)  # [N, 64]

    if "nc" not in _CACHE:
        _CACHE["nc"] = _build()
    nc = _CACHE["nc"]

    third = np.float32(1.0 / 3.0)
    iota = np.tile(np.arange(128, dtype=np.float32)[None, :],
                   (128, 1)).astype(ml_dtypes.bfloat16)
    # W_text staged as lhsT chunks [128, 3*64] bf16, pre-divided by 3
    wt_st = np.zeros((128, 3 * EMB_DIM), np.float32)
    for k in range(3):
        wt_st[:, k * EMB_DIM:(k + 1) * EMB_DIM] = W_text[k * 128:(k + 1) * 128, :]
    wt_st = (wt_st * third).astype(ml_dtypes.bfloat16)
    bt_st = np.tile((b_text * third)[None, :], (128, 1)).astype(ml_dtypes.bfloat16)
    wf_st = W_fuse.astype(ml_dtypes.bfloat16)
    bf_col = b_fuse[:, None].astype(np.float32)

    cores = _preprocess(edge_row, edge_col, edge_val, tail_mask, amp)

    in_maps = []
    for m in range(NCORES):
        pre = cores[m]
        lo = m * SHARD
        pi_l = pre["pi_l"]
        text_p = np.zeros((SHARD_P, TEXT_DIM), ml_dtypes.bfloat16)
        text_p[pi_l] = text_feats[lo:lo + SHARD].astype(ml_dtypes.bfloat16)
        id_p = np.zeros((SHARD_P, EMB_DIM), np.float32)
        id_p[pi_l] = emb_id[lo:lo + SHARD] * third
        in_maps.append({
            "text_T": np.ascontiguousarray(text_p.T),
            "id_b": id_p.astype(ml_dtypes.bfloat16),
            "gidx": pre["gidx"], "auxb": pre["auxb"], "auxv": pre["auxv"],
            "tailf_d": pre["tailf"],
            "w_text": wt_st, "b_text": bt_st,
            "w_fuse": wf_st, "b_fuse": bf_col,
            "iota_d": iota,
        })

    global _LAST_IN_MAPS
    _LAST_IN_MAPS = in_maps
    res = bass_utils.run_bass_kernel_spmd(nc, in_maps, core_ids=list(range(NCORES)))

    out = np.empty((N_NODES, EMB_DIM), np.float32)
    for m in range(NCORES):
        lo = m * SHARD
        out[lo:lo + SHARD] = res.results[m]["out"][cores[m]["pi_l"]]
    return out


# revision 28
# speedup vs baseline: 1.0009x; 1.0009x over previous
"""MDGRec GNN message-passing kernel for 8 Trainium2 NeuronCores.

Strategy (SPMD, one NEFF on 8 cores):
  - Nodes row-sharded: core m owns dst rows [m*18750, (m+1)*18750).
  - Host relabels nodes with a permutation pi so that each core's bin-packed
    128-row groups occupy contiguous rows of a padded 19200-row shard; all
    device-side writes/reads become contiguous slice DMAs.
  - id and text features concatenated into 128-wide rows; the 1/(N_LAYERS+1)
    layer-mean factor is folded into the staged id/text weights so every
    propagated term is pre-divided by 3.
  - Layer tables (full [153600, 128] bf16 in pi-space) built via on-device
    AllGather between layers.
  - SpMM per layer: bulk dma_gather of h[edge_col] striped across the 4 SWDGE
    queues (descriptor generation parallelizes across Q7 core pairs), scatter
    matrices built as two fused wide DVE ops per supergroup (is_eq + val
    multiply over [128, 70, 128]), segment-sum via PE matmuls into PSUM.
  - Fused epilogue (tail amp, gate, blend) computed per supergroup in
    transposed space; h0/h1 re-added on the PE via identity matmuls.
"""

import os
import numpy as np
import ml_dtypes

import concourse.bass as bass
import concourse.bacc as bacc
import concourse.tile as tile
import concourse.mybir as mybir
from concourse import bass_utils, library_config
from concourse.masks import make_identity

# ---- problem constants (hardcoded per spec) ----
N_NODES = 150000
EMB_DIM = 64
TEXT_DIM = 384
NCORES = 8
SHARD = N_NODES // NCORES          # 18750 real rows per core
F = 2 * EMB_DIM                    # 128 concat feature width

# ---- template constants ----
G = 150                            # groups per core
S_G = 2                            # groups per supergroup
N_SG = G // S_G                    # 75
SHARD_P = G * 128                  # 19200 padded rows per core (pi-space)
TBL_ROWS = NCORES * SHARD_P        # 153600 pi-space nodes
N_RANGE = 5
RANGE_SIZE = TBL_ROWS // N_RANGE   # 30720 (int16-safe)
C_GR = 7                           # chunks per (group, range)
CPG = N_RANGE * C_GR               # 35 chunks per group
C_SG = S_G * CPG                   # 70 chunks per supergroup
CALL_CH = S_G * C_GR               # 14 chunks per gather call
CALL_IDX = CALL_CH * 128           # 1792 idxs per gather call
W16 = CALL_IDX // 16               # 112 idx columns per range
CAP_R = C_GR * 128                 # 896 edge capacity per (group, range)
PAD_SLOT = 999.0
N_QUEUES = int(os.environ.get("KV2_QUEUES", "4"))
NEGTRIM = os.environ.get("KV2_NEGTRIM", "0") == "1"
FUSED_S = os.environ.get("KV2_FUSED_S", "1") == "1"
ACT_MOD = int(os.environ.get("KV2_ACT_MOD", "2"))  # sg % ACT_MOD -> ACT val-mult
SINGLE_PACKET = os.environ.get("KV2_SP", "0") == "1"

_CACHE = {}
_LAST_IN_MAPS = None


# ======================================================================
# device program
# ======================================================================

def _build(single_core=False):
    fp32 = mybir.dt.float32
    bf16 = mybir.dt.bfloat16
    i16 = mybir.dt.int16

    do_collectives = not single_core
    nc = bacc.Bacc("TRN2", target_bir_lowering=False, debug=False,
                   num_devices=1 if single_core else NCORES,
                   num_swdge_queues=N_QUEUES,
                   dynamic_dma_scratch_size=int(os.environ.get("KV2_SCRATCH",
                                                               "32768")))

    # inputs (per core)
    text_T = nc.dram_tensor("text_T", [TEXT_DIM, SHARD_P], bf16, kind="ExternalInput")
    id_b = nc.dram_tensor("id_b", [SHARD_P, EMB_DIM], bf16, kind="ExternalInput")
    gidx = nc.dram_tensor("gidx", [N_SG, 128, N_RANGE * W16], i16,
                          kind="ExternalInput")
    auxb = nc.dram_tensor("auxb", [128, N_SG * C_SG], bf16,
                          kind="ExternalInput")
    auxv = nc.dram_tensor("auxv", [128, N_SG * C_SG], fp32,
                          kind="ExternalInput")
    tailf_d = nc.dram_tensor("tailf_d", [128, N_SG * S_G], fp32,
                             kind="ExternalInput")
    w_text = nc.dram_tensor("w_text", [128, 3 * EMB_DIM], bf16, kind="ExternalInput")
    b_text = nc.dram_tensor("b_text", [128, EMB_DIM], bf16, kind="ExternalInput")
    w_fuse = nc.dram_tensor("w_fuse", [F, EMB_DIM], bf16, kind="ExternalInput")
    b_fuse = nc.dram_tensor("b_fuse", [EMB_DIM, 1], fp32, kind="ExternalInput")
    iota_d = nc.dram_tensor("iota_d", [128, 128], bf16, kind="ExternalInput")

    out = nc.dram_tensor("out", [SHARD_P, EMB_DIM], fp32, kind="ExternalOutput")

    # internal DRAM
    cat_bf = nc.dram_tensor("cat_bf", [SHARD_P, F], bf16)
    h1_bf = nc.dram_tensor("h1_bf", [SHARD_P, F], bf16)
    table0 = nc.dram_tensor("table0", [TBL_ROWS, F], bf16, addr_space="Shared")
    table1 = nc.dram_tensor("table1", [TBL_ROWS, F], bf16, addr_space="Shared")

    with tile.TileContext(nc) as tc:
        nc.gpsimd.load_library(library_config.mlp)
        with (
            tc.tile_pool(name="const", bufs=1) as cpool,
            tc.tile_pool(name="sb", bufs=3) as sb,
            tc.tile_pool(name="gx", bufs=3) as gx,
            tc.tile_pool(name="xp", bufs=int(os.environ.get("KV2_XBUFS", "4"))) as xp,
            tc.tile_pool(name="sp", bufs=3) as spool,
            tc.tile_pool(name="ep", bufs=2) as ep,
            tc.tile_pool(name="psum", bufs=1, space="PSUM") as ps,
            tc.tile_pool(name="psproj", bufs=2, space="PSUM") as psj,
            tc.tile_pool(name="psacc", bufs=2, space="PSUM") as psa,
        ):
            # ---- constants ----
            iota_b = cpool.tile([128, 128], bf16, tag="iota")
            nc.sync.dma_start(iota_b[:], iota_d[:])
            ident = cpool.tile([128, 128], fp32, tag="ident")
            make_identity(nc, ident[:])
            identb = cpool.tile([128, 128], bf16, tag="identb")
            nc.vector.tensor_copy(identb[:], ident[:])
            wt_t = cpool.tile([128, 3 * EMB_DIM], bf16, tag="wt")
            nc.sync.dma_start(wt_t[:], w_text[:])
            bt_t = cpool.tile([128, EMB_DIM], bf16, tag="bt")
            nc.sync.dma_start(bt_t[:], b_text[:])
            wf_t = cpool.tile([128, EMB_DIM], bf16, tag="wf")
            nc.sync.dma_start(wf_t[:], w_fuse[:])
            bf_t = cpool.tile([EMB_DIM, 1], fp32, tag="bf")
            nc.sync.dma_start(bf_t[:], b_fuse[:])
            aux_t = cpool.tile([128, N_SG * C_SG], bf16, tag="aux")
            nc.sync.dma_start(aux_t[:], auxb[:])
            auxv_t = cpool.tile([128, N_SG * C_SG], fp32, tag="auxv")
            nc.sync.dma_start(auxv_t[:], auxv[:])
            tailf_t = cpool.tile([128, N_SG, S_G], fp32, tag="tailf")
            nc.sync.dma_start(tailf_t[:].rearrange("p a b -> p (a b)"), tailf_d[:])

            # ---- text projection + cat assembly (pi-layout, all bf16) ----
            for i in range(G):
                r0 = i * 128
                proj_ps = psj.tile([128, EMB_DIM], fp32, tag="mm")
                tx3 = sb.tile([128, 3, 128], bf16, tag="tx3")
                for k in range(3):
                    nc.sync.dma_start(tx3[:, k, :],
                                      text_T[k * 128:(k + 1) * 128, r0:r0 + 128])
                for k in range(3):
                    nc.tensor.matmul(proj_ps[:], lhsT=tx3[:, k, :],
                                     rhs=wt_t[:, k * EMB_DIM:(k + 1) * EMB_DIM],
                                     start=(k == 0), stop=(k == 2))
                catb = sb.tile([128, F], bf16, tag="catb")
                nc.sync.dma_start(catb[:, 0:EMB_DIM], id_b[r0:r0 + 128, :])
                nc.vector.tensor_tensor(out=catb[:, EMB_DIM:F],
                                        in0=proj_ps[:], in1=bt_t[:],
                                        op=mybir.AluOpType.add)
                nc.sync.dma_start(cat_bf[r0:r0 + 128, :], catb[:])

            # ---- AllGather h0 ----
            if do_collectives:
                nc.gpsimd.collective_compute(
                    "AllGather", mybir.AluOpType.bypass,
                    replica_groups=[list(range(NCORES))],
                    ins=[cat_bf[:]],
                    outs=[table0[:]],
                )

            # ---- SpMM layers ----
            nidx_reg = nc.gpsimd.to_reg(CALL_IDX)
            qc = 0
            for layer in (0, 1):
                table = table0 if layer == 0 else table1
                for sg in range(N_SG):
                    gi = gx.tile([128, N_RANGE * W16], i16, tag="gi")
                    nc.sync.dma_start(gi[:], gidx[sg, :, :])
                    if layer == 1:
                        h0t = sb.tile([128, S_G, F], bf16, tag="h0")
                        h1t = sb.tile([128, S_G, F], bf16, tag="h1")
                        for s in range(S_G):
                            r0 = (sg * S_G + s) * 128
                            nc.sync.dma_start(h0t[:, s, :],
                                              cat_bf[r0:r0 + 128, :])
                            nc.sync.dma_start(h1t[:, s, :],
                                              h1_bf[r0:r0 + 128, :])

                    Xsr = []
                    for r in range(N_RANGE):
                        X = xp.tile([128, CALL_CH, F], bf16, tag=f"X{r}")
                        nc.gpsimd.dma_gather(
                            X[:],
                            table[r * RANGE_SIZE:(r + 1) * RANGE_SIZE, :],
                            gi[:, r * W16:(r + 1) * W16], CALL_IDX, nidx_reg, F,
                            single_packet=SINGLE_PACKET, queue_num=qc % N_QUEUES)
                        qc += 1
                        Xsr.append(X)

                    # S[p, ci, j] = (iota[j] == slot[p, ci]) * val[p, ci]
                    a0 = sg * C_SG
                    slot_ap = aux_t[:, a0:a0 + C_SG]
                    val_ap = auxv_t[:, a0:a0 + C_SG]
                    S_t = spool.tile([128, C_SG, 128], bf16, tag="S")
                    nc.vector.tensor_tensor(
                        out=S_t[:],
                        in0=iota_b[:, None, :].broadcast_to([128, C_SG, 128]),
                        in1=slot_ap[:, :, None].broadcast_to([128, C_SG, 128]),
                        op=mybir.AluOpType.is_equal)
                    if sg % ACT_MOD == ACT_MOD - 1:
                        # val multiply on the (otherwise idle) scalar engine
                        for ci in range(C_SG):
                            nc.scalar.activation(
                                S_t[:, ci, :], S_t[:, ci, :],
                                mybir.ActivationFunctionType.Copy,
                                scale=val_ap[:, ci:ci + 1])
                    else:
                        nc.vector.tensor_tensor(
                            out=S_t[:],
                            in0=S_t[:],
                            in1=val_ap[:, :, None].broadcast_to([128, C_SG, 128]),
                            op=mybir.AluOpType.mult)

                    acc = psa.tile([128, S_G, F], fp32, tag="acc")
                    for s in range(S_G):
                        g = sg * S_G + s
                        r0 = g * 128
                        chunks = [(r, s * C_GR + c)
                                  for r in range(N_RANGE) for c in range(C_GR)]
                        n_mm = CPG + (2 if layer == 1 else 0)
                        for j, (r, k) in enumerate(chunks):
                            ci = r * CALL_CH + k
                            nc.tensor.matmul(acc[:, s, :], lhsT=S_t[:, ci, :],
                                             rhs=Xsr[r][:, k, :],
                                             start=(j == 0), stop=(j == n_mm - 1))
                        if layer == 1:
                            nc.tensor.matmul(acc[:, s, :], lhsT=identb[:],
                                             rhs=h0t[:, s, :], start=False,
                                             stop=False)
                            nc.tensor.matmul(acc[:, s, :], lhsT=identb[:],
                                             rhs=h1t[:, s, :], start=False,
                                             stop=True)

                    if layer == 0:
                        resb = sb.tile([128, S_G, F], bf16, tag="resb")
                        nc.scalar.activation(resb[:], acc[:],
                                             mybir.ActivationFunctionType.Copy)
                        for s in range(S_G):
                            r0 = (sg * S_G + s) * 128
                            nc.sync.dma_start(h1_bf[r0:r0 + 128, :],
                                              resb[:, s, :])
                        continue

                    # ---- fused epilogue for this supergroup (node-major) ----
                    fs = ep.tile([128, S_G, F], fp32, tag="fs")
                    nc.vector.tensor_copy(fs[:], acc[:])
                    # amp on text half, per-node (per-partition) multiplier
                    nc.vector.tensor_tensor(
                        out=fs[:, :, EMB_DIM:F],
                        in0=fs[:, :, EMB_DIM:F],
                        in1=tailf_t[:, sg, :, None].broadcast_to(
                            [128, S_G, EMB_DIM]),
                        op=mybir.AluOpType.mult)
                    # transpose fsum for the gate matmul
                    tp = ps.tile([128, S_G, 128], fp32, tag="tp")
                    for s in range(S_G):
                        nc.tensor.transpose(out=tp[:, s, :], in_=fs[:, s, :],
                                            identity=ident[:])
                    ft = ep.tile([128, S_G, 128], bf16, tag="ft")
                    nc.vector.tensor_copy(ft[:], tp[:])
                    gp = ps.tile([EMB_DIM, S_G * 128], fp32, tag="gp")
                    nc.tensor.matmul(gp[:], lhsT=wf_t[:],
                                     rhs=ft[:].rearrange("p s n -> p (s n)"),
                                     start=True, stop=True)
                    gateT = ep.tile([EMB_DIM, S_G * 128], fp32, tag="gateT")
                    nc.scalar.activation(gateT[:], gp[:],
                                         mybir.ActivationFunctionType.Sigmoid,
                                         bias=bf_t[:, :1])
                    g2 = ps.tile([128, S_G, EMB_DIM], fp32, tag="g2")
                    for s in range(S_G):
                        nc.tensor.transpose(out=g2[:, s, :],
                                            in_=gateT[:, s * 128:(s + 1) * 128],
                                            identity=ident[0:EMB_DIM, 0:EMB_DIM])
                    gate = ep.tile([128, S_G, EMB_DIM], fp32, tag="gate")
                    nc.vector.tensor_copy(gate[:], g2[:])
                    # blend: fused = text + gate*(id - text)
                    dif = ep.tile([128, S_G, EMB_DIM], fp32, tag="dif")
                    nc.vector.tensor_tensor(out=dif[:], in0=fs[:, :, 0:EMB_DIM],
                                            in1=fs[:, :, EMB_DIM:F],
                                            op=mybir.AluOpType.subtract)
                    nc.vector.tensor_tensor(out=dif[:], in0=dif[:], in1=gate[:],
                                            op=mybir.AluOpType.mult)
                    res = ep.tile([128, S_G, EMB_DIM], fp32, tag="res")
                    nc.vector.tensor_tensor(out=res[:], in0=fs[:, :, EMB_DIM:F],
                                            in1=dif[:], op=mybir.AluOpType.add)
                    for s in range(S_G):
                        r0 = (sg * S_G + s) * 128
                        nc.sync.dma_start(out[r0:r0 + 128, :], res[:, s, :])

                if layer == 0 and do_collectives:
                    nc.gpsimd.collective_compute(
                        "AllGather", mybir.AluOpType.bypass,
                        replica_groups=[list(range(NCORES))],
                        ins=[h1_bf[:]],
                        outs=[table1[:]],
                    )

    nc.compile()
    return nc


# ======================================================================
# host preprocessing
# ======================================================================

def _preprocess(edge_row, edge_col, edge_val, tail_mask, amp):
    """Full host-side preprocessing. Two passes:
    1. per-core packing of dst rows into groups by total degree (snake on
       sorted degrees), defining the pi permutation; then exact per-
       (group, src-range) bucket counts are checked against CAP_R and
       repaired by moving rows between groups.
    2. per-core edge template fill (gather indices, slots, values)."""
    # ---- pass 1: pack by total degree, then repair ----
    grp = np.empty(N_NODES, np.int64)
    slot = np.empty(N_NODES, np.int64)
    deg_t = np.bincount(edge_row, minlength=N_NODES)
    for m in range(NCORES):
        lo = m * SHARD
        dt_ = deg_t[lo:lo + SHARD]
        order = np.argsort(-dt_, kind="stable")
        gassign = np.empty(SHARD, np.int64)
        idx = np.arange(SHARD)
        rounds = idx // G
        posr = idx % G
        fwd = (rounds % 2 == 0)
        gassign[order] = np.where(fwd, posr, G - 1 - posr)
        grp[lo:lo + SHARD] = gassign
        o2 = np.lexsort((np.arange(SHARD), gassign))
        sg_sorted = gassign[o2]
        starts = np.searchsorted(sg_sorted, np.arange(G))
        sl = np.arange(SHARD) - starts[sg_sorted]
        slot_l = np.empty(SHARD, np.int64)
        slot_l[o2] = sl
        slot[lo:lo + SHARD] = slot_l

    pi = ((np.arange(N_NODES) // SHARD) * SHARD_P + grp * 128 + slot)

    # ---- check/repair (group, range) capacities per core ----
    pc = pi[edge_col]
    rng_id = pc // RANGE_SIZE
    for m in range(NCORES):
        lo = m * SHARD
        sel = (edge_row >= lo) & (edge_row < lo + SHARD)
        er = edge_row[sel] - lo
        rr = rng_id[sel]
        gg = grp[lo + er]
        for _ in range(50):
            cnt = np.zeros((G, N_RANGE), np.int64)
            np.add.at(cnt, (gg, rr), 1)
            over = np.argwhere(cnt > CAP_R)
            if len(over) == 0:
                break
            nrows = np.bincount(grp[lo:lo + SHARD], minlength=G)
            deg_gr = np.zeros((SHARD, N_RANGE), np.int64)
            np.add.at(deg_gr, (er, rr), 1)
            for g_o, r_o in over:
                rows_g = np.where(grp[lo:lo + SHARD] == g_o)[0]
                rows_g = rows_g[np.argsort(-deg_gr[rows_g, r_o])]
                moved = False
                need = cnt[g_o, r_o] - CAP_R
                for row in rows_g:
                    if deg_gr[row, r_o] == 0:
                        break
                    for g_n in np.argsort(cnt[:, r_o]):
                        if g_n == g_o or nrows[g_n] >= 128:
                            continue
                        if np.all(cnt[g_n] + deg_gr[row] <= CAP_R):
                            cnt[g_o] -= deg_gr[row]
                            cnt[g_n] += deg_gr[row]
                            nrows[g_o] -= 1
                            nrows[g_n] += 1
                            grp[lo + row] = g_n
                            gg = grp[lo + er]
                            moved = True
                            break
                    need = cnt[g_o, r_o] - CAP_R
                    if need <= 0:
                        break
                if not moved and cnt[g_o, r_o] > CAP_R:
                    raise RuntimeError("capacity repair failed")
            gassign = grp[lo:lo + SHARD]
            o2 = np.lexsort((np.arange(SHARD), gassign))
            sg_sorted = gassign[o2]
            starts = np.searchsorted(sg_sorted, np.arange(G))
            sl = np.arange(SHARD) - starts[sg_sorted]
            slot_l = np.empty(SHARD, np.int64)
            slot_l[o2] = sl
            slot[lo:lo + SHARD] = slot_l
        else:
            raise RuntimeError("repair loop did not converge")
        pi = (np.arange(N_NODES) // SHARD) * SHARD_P + grp * 128 + slot
        pc = pi[edge_col]
        rng_id = pc // RANGE_SIZE

    # ---- pass 2: per-core template fill ----
    cores = []
    for m in range(NCORES):
        lo = m * SHARD
        sel = (edge_row >= lo) & (edge_row < lo + SHARD)
        er = edge_row[sel] - lo
        ev = edge_val[sel].astype(np.float32)
        e_pc = pc[sel]                      # pi-space col
        e_r = (e_pc // RANGE_SIZE).astype(np.int64)
        e_cloc = (e_pc - e_r * RANGE_SIZE).astype(np.int64)
        e_g = grp[lo + er]
        e_slot = slot[lo + er]

        bucket = e_g * N_RANGE + e_r
        eorder = np.argsort(bucket, kind="stable")
        b_sorted = bucket[eorder]
        cnt = np.bincount(b_sorted, minlength=G * N_RANGE)
        assert cnt.max() <= CAP_R, cnt.max()
        off = np.zeros(G * N_RANGE + 1, np.int64)
        np.cumsum(cnt, out=off[1:])
        pos = np.arange(len(eorder)) - off[b_sorted]

        so_g = e_g[eorder]
        so_r = e_r[eorder]
        e_sg = so_g // S_G
        e_s = so_g % S_G
        e_c = pos // 128
        e_p = pos % 128
        e_ci = so_r * CALL_CH + e_s * C_GR + e_c

        # slot aux (bf16) and val aux (fp32): [128, N_SG * C_SG]
        slot_arr = np.full((128, N_SG, C_SG), PAD_SLOT, np.float32)
        val_arr = np.zeros((128, N_SG, C_SG), np.float32)
        lin = (e_p * N_SG + e_sg) * C_SG + e_ci
        slot_arr.reshape(-1)[lin] = e_slot[eorder].astype(np.float32)
        val_arr.reshape(-1)[lin] = ev[eorder]
        aux_bf = slot_arr.reshape(128, -1).astype(ml_dtypes.bfloat16)
        aux_v = val_arr.reshape(128, -1)

        # gather indices, wrapped [16, W16] and replicated to 128 partitions;
        # tail of each (sg, range) call marked -1 (ucode trims trailing negs)
        gidx16 = np.zeros((N_SG, N_RANGE, 16, W16), np.int16)
        e_k = e_s * C_GR + e_c
        q = e_k * 128 + e_p
        lin2 = ((e_sg * N_RANGE + so_r) * 16 + (q % 16)) * W16 + (q // 16)
        gidx16.reshape(-1)[lin2] = e_cloc[eorder].astype(np.int16)
        if NEGTRIM:
            # mark trailing unused q positions of each call as -1
            qmax = np.zeros((N_SG, N_RANGE), np.int64)
            np.maximum.at(qmax, (e_sg, so_r), q + 1)
            qgrid = np.arange(CALL_IDX).reshape(16, W16, order="F")
            trail = qgrid[None, None, :, :] >= qmax[:, :, None, None]
            gidx16[trail] = -1
        gidx_arr = np.tile(gidx16, (1, 1, 8, 1))          # [N_SG, 5, 128, W16]
        gidx_arr = np.ascontiguousarray(
            gidx_arr.transpose(0, 2, 1, 3).reshape(N_SG, 128, -1))

        pi_l = grp[lo:lo + SHARD] * 128 + slot[lo:lo + SHARD]  # local padded pos
        # tail factor per node (amp or 1), node-major layout [128, N_SG*S_G]
        tf_p = np.full(SHARD_P, 1.0, np.float32)
        tmask = tail_mask[lo:lo + SHARD].astype(bool)
        tf_p[pi_l] = np.where(tmask, amp, 1.0).astype(np.float32)
        # tf_p index = (sg*S_G+s)*128 + p  ->  [p, sg*S_G+s]
        tailf = np.ascontiguousarray(tf_p.reshape(N_SG * S_G, 128).T)

        cores.append({
            "gidx": gidx_arr, "auxb": aux_bf, "auxv": aux_v, "tailf": tailf,
            "pi_l": pi_l,
        })
    return cores


def kernel(text_feats, edge_row, edge_col, edge_val, tail_mask, user_emb,
           item_emb, W_text, b_text, W_fuse, b_fuse, tail_amp):
    text_feats = np.asarray(text_feats, np.float32)
    edge_row = np.asarray(edge_row).astype(np.int64)
    edge_col = np.asarray(edge_col).astype(np.int64)
    edge_val = np.asarray(edge_val, np.float32)
    tail_mask = np.asarray(tail_mask).astype(bool)
    user_emb = np.asarray(user_emb, np.float32)
    item_emb = np.asarray(item_emb, np.float32)
    W_text = np.asarray(W_text, np.float32)
    b_text = np.asarray(b_text, np.float32)
    W_fuse = np.asarray(W_fuse, np.float32)
    b_fuse = np.asarray(b_fuse, np.float32)
    amp = float(1.0 + 1.0 / (1.0 + np.exp(-np.float64(np.asarray(tail_amp)))))

    emb_id = np.concatenate([user_emb, item_emb], axis=0)  # [N, 64]

    if "nc" not in _CACHE:
        _CACHE["nc"] = _build()
    nc = _CACHE["nc"]

    third = np.float32(1.0 / 3.0)
    iota = np.tile(np.arange(128, dtype=np.float32)[None, :],
                   (128, 1)).astype(ml_dtypes.bfloat16)
    # W_text staged as lhsT chunks [128, 3*64] bf16, pre-divided by 3
    wt_st = np.zeros((128, 3 * EMB_DIM), np.float32)
    for k in range(3):
        wt_st[:, k * EMB_DIM:(k + 1) * EMB_DIM] = W_text[k * 128:(k + 1) * 128, :]
    wt_st = (wt_st * third).astype(ml_dtypes.bfloat16)
    bt_st = np.tile((b_text * third)[None, :], (128, 1)).astype(ml_dtypes.bfloat16)
    wf_st = W_fuse.astype(ml_dtypes.bfloat16)
    bf_col = b_fuse[:, None].astype(np.float32)

    cores = _preprocess(edge_row, edge_col, edge_val, tail_mask, amp)

    in_maps = []
    for m in range(NCORES):
        pre = cores[m]
        lo = m * SHARD
        pi_l = pre["pi_l"]
        text_p = np.zeros((SHARD_P, TEXT_DIM), ml_dtypes.bfloat16)
        text_p[pi_l] = text_feats[lo:lo + SHARD].astype(ml_dtypes.bfloat16)
        id_p = np.zeros((SHARD_P, EMB_DIM), np.float32)
        id_p[pi_l] = emb_id[lo:lo + SHARD] * third
        in_maps.append({
            "text_T": np.ascontiguousarray(text_p.T),
            "id_b": id_p.astype(ml_dtypes.bfloat16),
            "gidx": pre["gidx"], "auxb": pre["auxb"], "auxv": pre["auxv"],
            "tailf_d": pre["tailf"],
            "w_text": wt_st, "b_text": bt_st,
            "w_fuse": wf_st, "b_fuse": bf_col,
            "iota_d": iota,
        })

    global _LAST_IN_MAPS
    _LAST_IN_MAPS = in_maps
    res = bass_utils.run_bass_kernel_spmd(nc, in_maps, core_ids=list(range(NCORES)))

    out = np.empty((N_NODES, EMB_DIM), np.float32)
    for m in range(NCORES):
        lo = m * SHARD
        out[lo:lo + SHARD] = res.results[m]["out"][cores[m]["pi_l"]]
    return out
